# revision 1
# baseline (speedup 1.0000x reference)
"""Causal self-attention (B=2, S=2048, D=2048, H=16) on 8 TRN2 NeuronCores.

Sharding (data + tensor parallel, per the head-group hint):
  core c -> batch b = c // 4, head group g = c % 4 (heads 4g..4g+3).
  wq/wk/wv are split column-wise per head group (512 cols), wo row-wise
  (512 rows). Each core computes attention for its 4 heads on its batch and
  produces a partial output projection; the host sums the 4 partials per
  batch (the tensor-parallel all-reduce, done at gather time).

Device kernel layout trick: all activations are kept "transposed"
(feature-major) so every matmul consumes operands in their natural layout
and no on-device transpose is ever needed:
  QT[c,s] = wq.T @ x.T          (lhsT=wq,  rhs=xT      — both native)
  KT[c,s] = wk.T @ x.T
  V[s,c]  = x @ wv              (lhsT=xT,  rhs=wv      — both native)
  ST[k,q] = K_h Q_h^T           (lhsT=KT_h, rhs=QT_h)
  PT[k,q] = exp(ST*scale - 4 + causal_mask)             (ACT engine)
  OT[hd,q]= V_h.T @ PT          (lhsT=V_h, rhs=PT)      accumulated in PSUM
  rsum[q] = ones.T @ PT         (softmax denominator, PE ones-matmul)
  out     = (OT/rsum).T @ wo    (lhsT=OT,  rhs=wo)
Compute dtype fp16 (measured matmul rel-err ~3e-4, well under the fp32
envelope gate); softmax statistics and all PSUM accumulation in fp32.
"""

import math

import numpy as np

B = 2
S = 2048
D = 2048
H = 16
HD = 128
N_CORES = 8
NH = 4          # heads per core
C = NH * HD     # 512 per-core projection width
P = 128
DO = D // P     # 16 contraction subtiles
SBLK = 512      # matmul moving free dim / PSUM bank
NSB = S // SBLK  # 4 sequence blocks
NKB = S // P     # 16 key blocks
SCALE = 1.0 / math.sqrt(HD)
EBIAS = -4.0    # constant shift inside exp; cancels in softmax ratio
MASK_NEG = -1e9

_STATE = {}


def _build_kernel(repeat=1):
    import concourse.bacc as bacc
    import concourse.mybir as mybir
    import concourse.tile as tile
    from concourse.bass import ts

    F16 = mybir.dt.float16
    F32 = mybir.dt.float32

    nc = bacc.Bacc("TRN2", target_bir_lowering=False, debug=False)

    xt_d = nc.dram_tensor("xt", [D, S], F16, kind="ExternalInput").ap()
    wq_d = nc.dram_tensor("wq", [D, C], F16, kind="ExternalInput").ap()
    wk_d = nc.dram_tensor("wk", [D, C], F16, kind="ExternalInput").ap()
    wv_d = nc.dram_tensor("wv", [D, C], F16, kind="ExternalInput").ap()
    wo_d = nc.dram_tensor("wo", [C, D], F16, kind="ExternalInput").ap()
    out_d = nc.dram_tensor("out", [S, D], F32, kind="ExternalOutput").ap()

    with tile.TileContext(nc) as tc:
        with tc.tile_pool(name="persist", bufs=1) as p_per:
            ot = p_per.tile([P, NH, S], F16)      # normalized attn out^T
            qt = p_per.tile([P, NH, S], F16)
            kt = p_per.tile([P, NH, S], F16)
            v = p_per.tile([P, DO, C], F16)
            masks = p_per.tile([P, NH, SBLK], F32)
            ones = p_per.tile([P, P], F16)
            ebias = p_per.tile([P, 1], F32)

            nc.gpsimd.memset(ones[:], 1.0)
            nc.gpsimd.memset(ebias[:], EBIAS)
            for a in range(4):
                nc.gpsimd.memset(masks[:, a, :], 0.0)
                # keep (j - p - 128a >= 0) i.e. k_global <= q_global
                nc.gpsimd.affine_select(
                    out=masks[:, a, :],
                    in_=masks[:, a, :],
                    compare_op=mybir.AluOpType.is_ge,
                    fill=MASK_NEG,
                    base=-(a * P),
                    channel_multiplier=-1,
                    pattern=[[1, SBLK]],
                )

            # ---------------- Phase 1: QKV projections ----------------
            for _rep in range(repeat):
              with tc.tile_pool(name="xw", bufs=1) as p_xw, \
                   tc.tile_pool(name="p1ps", bufs=6, space="PSUM") as p1ps:
                  xt_r = xt_d.rearrange("(do p) s -> do p s", p=P)
                  xts = []
                  for do in range(DO):
                      t = p_xw.tile([P, S], F16, tag=f"xt{do}", name=f"xt{do}")
                      # alternate the two HWDGE engines for queue parallelism
                      eng = nc.sync if do % 2 == 0 else nc.scalar
                      eng.dma_start(t[:], xt_r[do])
                      xts.append(t)
                  wq_sb = p_xw.tile([P, DO, C], F16, tag="wq")
                  wk_sb = p_xw.tile([P, DO, C], F16, tag="wk")
                  wv_sb = p_xw.tile([P, DO, C], F16, tag="wv")
                  # chunk weight loads by 4 d-subtiles so the first matmul
                  # rounds start after 512 KB, not after the full 2 MB
                  wq_r = wq_d.rearrange("(do p) c -> p do c", p=P)
                  wk_r = wk_d.rearrange("(do p) c -> p do c", p=P)
                  wv_r = wv_d.rearrange("(do p) c -> p do c", p=P)
                  for dc in range(0, DO, 4):
                      sl = slice(dc, dc + 4)
                      nc.scalar.dma_start(wq_sb[:, sl, :], wq_r[:, sl, :])
                      nc.sync.dma_start(wk_sb[:, sl, :], wk_r[:, sl, :])
                      nc.scalar.dma_start(wv_sb[:, sl, :], wv_r[:, sl, :])

                  # 48 accumulation groups, st-major with V interleaved so
                  # the tiles phase 2 needs first (st=0 QT/KT rows + low-kb
                  # V rows) are produced first, and each xt subtile is
                  # consumed right after its DMA completes.
                  groups = []
                  for st in range(NSB):
                      for ct in range(NH):
                          groups.append(("q", ct, st))
                          groups.append(("k", ct, st))
                      for sv in range(4 * st, 4 * st + 4):
                          groups.append(("v", sv, 0))

                  GCHUNK = 3
                  for gstart in range(0, len(groups), GCHUNK):
                      chunk = groups[gstart:gstart + GCHUNK]
                      psums = []
                      for kind, i0, i1 in chunk:
                          psums.append(p1ps.tile([P, SBLK], F32, tag="p1", name="p1ps"))
                      for do in range(DO):
                          for gi, (kind, i0, i1) in enumerate(chunk):
                              first = do == 0
                              last = do == DO - 1
                              if kind == "q":
                                  nc.tensor.matmul(
                                      psums[gi][:],
                                      wq_sb[:, do, ts(i0, P)],
                                      xts[do][:, ts(i1, SBLK)],
                                      start=first, stop=last)
                              elif kind == "k":
                                  nc.tensor.matmul(
                                      psums[gi][:],
                                      wk_sb[:, do, ts(i0, P)],
                                      xts[do][:, ts(i1, SBLK)],
                                      start=first, stop=last)
                              else:
                                  nc.tensor.matmul(
                                      psums[gi][:],
                                      xts[do][:, ts(i0, P)],
                                      wv_sb[:, do, :],
                                      start=first, stop=last)
                      for gi, (kind, i0, i1) in enumerate(chunk):
                          if kind == "q":
                              nc.any.tensor_copy(qt[:, i0, ts(i1, SBLK)], psums[gi][:])
                          elif kind == "k":
                              nc.any.tensor_copy(kt[:, i0, ts(i1, SBLK)], psums[gi][:])
                          else:
                              nc.any.tensor_copy(v[:, i0, :], psums[gi][:])

              # ---------------- Phase 2: causal attention ----------------
              with tc.tile_pool(name="p2w", bufs=4) as p2w, \
                   tc.tile_pool(name="p2stat", bufs=2) as p2stat, \
                   tc.tile_pool(name="ps_s", bufs=4, space="PSUM") as ps_s, \
                   tc.tile_pool(name="ps_av", bufs=2, space="PSUM") as ps_av, \
                   tc.tile_pool(name="ps_rs", bufs=2, space="PSUM") as ps_rs:
                  for qb in range(NSB):
                      nkb = 4 * (qb + 1)  # causal: only key blocks <= q block
                      for h in range(NH):
                          av = ps_av.tile([P, SBLK], F32, tag="av")
                          rs = ps_rs.tile([P, SBLK], F32, tag="rs")
                          for kb in range(nkb):
                              sc = ps_s.tile([P, SBLK], F32, tag="sc")
                              nc.tensor.matmul(
                                  sc[:],
                                  kt[:, h, ts(kb, P)],
                                  qt[:, h, ts(qb, SBLK)],
                                  start=True, stop=True)
                              if kb >= nkb - 4:
                                  a = kb - 4 * qb
                                  tmp = p2w.tile([P, SBLK], F32, tag="msk")
                                  nc.vector.tensor_add(tmp[:], sc[:], masks[:, a, :])
                                  src = tmp
                              else:
                                  src = sc
                              probs = p2w.tile([P, SBLK], F16, tag="probs")
                              nc.scalar.activation(
                                  probs[:], src[:],
                                  mybir.ActivationFunctionType.Exp,
                                  bias=ebias[:], scale=SCALE)
                              nc.tensor.matmul(
                                  av[:],
                                  v[:, kb, ts(h, P)],
                                  probs[:],
                                  start=(kb == 0), stop=(kb == nkb - 1))
                              nc.tensor.matmul(
                                  rs[:],
                                  ones[:],
                                  probs[:],
                                  start=(kb == 0), stop=(kb == nkb - 1))
                          rcp = p2stat.tile([P, SBLK], F32, tag="rcp")
                          nc.vector.reciprocal(rcp[:], rs[:])
                          nc.vector.tensor_tensor(
                              ot[:, h, ts(qb, SBLK)], av[:], rcp[:],
                              op=mybir.AluOpType.mult)

              # ---------------- Phase 3: output projection ----------------
              with tc.tile_pool(name="p3w", bufs=1) as p3w, \
                   tc.tile_pool(name="p3stage", bufs=4) as p3stage, \
                   tc.tile_pool(name="p3ps", bufs=4, space="PSUM") as p3ps:
                  wo_sb = p3w.tile([P, NH, D], F16, tag="wo")
                  nc.sync.dma_start(wo_sb[:], wo_d.rearrange("(cs p) d -> p cs d", p=P))
                  for so in range(NKB):
                      for no in range(NSB):
                          po = p3ps.tile([P, SBLK], F32, tag="po")
                          for cs in range(NH):
                              nc.tensor.matmul(
                                  po[:],
                                  ot[:, cs, ts(so, P)],
                                  wo_sb[:, cs, ts(no, SBLK)],
                                  start=(cs == 0), stop=(cs == NH - 1))
                          stage = p3stage.tile([P, SBLK], F32, tag="st")
                          nc.any.tensor_copy(stage[:], po[:])
                          nc.sync.dma_start(
                              out_d[ts(so, P), ts(no, SBLK)], stage[:])

    nc.compile()
    return nc


def _shard_inputs(x, wq, wk, wv, wo):
    in_maps = []
    for c in range(N_CORES):
        b, g = divmod(c, NH)
        cols = slice(g * C, (g + 1) * C)
        in_maps.append({
            "xt": np.ascontiguousarray(x[b].T).astype(np.float16),
            "wq": wq[:, cols].astype(np.float16),
            "wk": wk[:, cols].astype(np.float16),
            "wv": wv[:, cols].astype(np.float16),
            "wo": np.ascontiguousarray(wo[cols, :]).astype(np.float16),
        })
    return in_maps


def kernel(x, wq, wk, wv, wo):
    from concourse.bass_utils import run_bass_kernel_spmd

    if "nc" not in _STATE:
        _STATE["nc"] = _build_kernel()
    nc = _STATE["nc"]

    in_maps = _shard_inputs(
        np.asarray(x), np.asarray(wq), np.asarray(wk),
        np.asarray(wv), np.asarray(wo))
    res = run_bass_kernel_spmd(nc, in_maps, core_ids=list(range(N_CORES)))
    out = np.zeros((B, S, D), dtype=np.float32)
    for c in range(N_CORES):
        b = c // NH
        out[b] += res.results[c]["out"]
    return out



# revision 21
# speedup vs baseline: 1.2385x; 1.2385x over previous
"""Causal self-attention (B=2, S=2048, D=2048, H=16) on 8 TRN2 NeuronCores.

Sharding (data + tensor parallel, per the head-group hint):
  core c -> batch b = c // 4, head group g = c % 4 (heads 4g..4g+3).
  wq/wk/wv are split column-wise per head group (512 cols), wo row-wise
  (512 rows). Each core computes attention for its 4 heads on its batch and
  produces a partial output projection; the host sums the 4 partials per
  batch (the tensor-parallel all-reduce, done at gather time).

Device kernel layout trick: all activations are kept "transposed"
(feature-major) so every matmul consumes operands in their natural layout
and no on-device transpose is ever needed:
  QT[c,s] = wq.T @ x.T          (lhsT=wq,  rhs=xT      — both native)
  KT[c,s] = wk.T @ x.T
  V[s,c]  = x @ wv              (lhsT=xT,  rhs=wv      — both native)
  ST[k,q] = K_h Q_h^T           (lhsT=KT_h, rhs=QT_h)
  PT[k,q] = exp(ST*scale - 4 + causal_mask)             (ACT engine)
  OT[hd,q]= V_h.T @ PT          (lhsT=V_h, rhs=PT)      accumulated in PSUM
  rsum[q] = ones.T @ PT         (softmax denominator, PE ones-matmul)
  out     = (OT/rsum).T @ wo    (lhsT=OT,  rhs=wo)
Compute dtype fp16 (measured matmul rel-err ~3e-4, well under the fp32
envelope gate); softmax statistics and all PSUM accumulation in fp32.

Scheduling (v2):
  - Phase-1 inputs staged over 4 DMA queues, issue-ordered so the first
    matmul group's operands (wq/wk chunk 0, xt0/xt1) are queue heads.
  - wo prefetched during phase 1 (SBUF peak 200/208 KB per partition).
  - Phase 2 software-pipelined: scores matmul for step i+1 issues before
    the AV/rsum matmuls of step i, hiding the ACT exp latency.
  - Output partials written fp16 (host accumulates in fp32).
"""

import math

import numpy as np

B = 2
S = 2048
D = 2048
H = 16
HD = 128
N_CORES = 8
NH = 4          # heads per core
C = NH * HD     # 512 per-core projection width
P = 128
DO = D // P     # 16 contraction subtiles
SBLK = 512      # matmul moving free dim / PSUM bank
NSB = S // SBLK  # 4 sequence blocks
NKB = S // P     # 16 key blocks
SCALE = 1.0 / math.sqrt(HD)
EBIAS = -4.0    # constant shift inside exp; cancels in softmax ratio
MASK_NEG = -1e9

_STATE = {}


def _build_kernel(repeat=1):
    import concourse.bacc as bacc
    import concourse.mybir as mybir
    import concourse.tile as tile
    from concourse.bass import ts

    F16 = mybir.dt.float16
    F32 = mybir.dt.float32

    nc = bacc.Bacc("TRN2", target_bir_lowering=False, debug=False)

    xt_d = nc.dram_tensor("xt", [D, S], F16, kind="ExternalInput").ap()
    wq_d = nc.dram_tensor("wq", [D, C], F16, kind="ExternalInput").ap()
    wk_d = nc.dram_tensor("wk", [D, C], F16, kind="ExternalInput").ap()
    wv_d = nc.dram_tensor("wv", [D, C], F16, kind="ExternalInput").ap()
    wo_d = nc.dram_tensor("wo", [C, D], F16, kind="ExternalInput").ap()
    out_d = nc.dram_tensor("out", [S, D], F16, kind="ExternalOutput").ap()

    with tile.TileContext(nc) as tc:
        with tc.tile_pool(name="persist", bufs=1) as p_per:
            # kt split per head, v per kb-quadrant, ot per q-block: tile
            # dependencies are tracked per-tile, so consumers at phase
            # boundaries must not falsely wait on the last writer of an
            # unrelated slice
            qt = p_per.tile([P, NH, S], F16)
            kts = [p_per.tile([P, S], F16, name=f"kt{h}") for h in range(NH)]
            vs = [p_per.tile([P, 4, C], F16, name=f"v{m}") for m in range(4)]
            ots = [p_per.tile([P, NH, SBLK], F16, name=f"ot{qb}")
                   for qb in range(NSB)]
            masks = p_per.tile([P, NH, SBLK], F32)
            ones = p_per.tile([P, P], F16)
            ebias = p_per.tile([P, 1], F32)

            nc.gpsimd.memset(ones[:], 1.0)
            nc.gpsimd.memset(ebias[:], EBIAS)
            for a in range(4):
                nc.gpsimd.memset(masks[:, a, :], 0.0)
                # keep (j - p - 128a >= 0) i.e. k_global <= q_global
                nc.gpsimd.affine_select(
                    out=masks[:, a, :],
                    in_=masks[:, a, :],
                    compare_op=mybir.AluOpType.is_ge,
                    fill=MASK_NEG,
                    base=-(a * P),
                    channel_multiplier=-1,
                    pattern=[[1, SBLK]],
                )

            for _rep in range(repeat):
              with tc.tile_pool(name="p3w", bufs=1) as p3w:
                # ---------------- Phase 1: QKV projections ----------------
                with tc.tile_pool(name="xw", bufs=1) as p_xw, \
                     tc.tile_pool(name="p1ps", bufs=8, space="PSUM") as p1ps:
                    xt_r = xt_d.rearrange("(do p) s -> do p s", p=P)
                    xts = [p_xw.tile([P, S], F16, tag=f"xt{do}", name=f"xt{do}")
                           for do in range(DO)]
                    wq_sb = p_xw.tile([P, DO, C], F16, tag="wq")
                    wk_sb = p_xw.tile([P, DO, C], F16, tag="wk")
                    wv_sb = p_xw.tile([P, DO, C], F16, tag="wv")
                    wo_sb = p3w.tile([P, NH, D], F16, tag="wo")
                    wq_r = wq_d.rearrange("(do p) c -> p do c", p=P)
                    wk_r = wk_d.rearrange("(do p) c -> p do c", p=P)
                    wv_r = wv_d.rearrange("(do p) c -> p do c", p=P)
                    # The DMA engines drain the two HWDGE queues round-robin
                    # into one ~350 GB/s serial pipe, so global issue order
                    # == arrival order, and each dma_start costs ~1.3 us of
                    # issuing-sequencer time (so keep transfers >= 512 KB).
                    # Groups are ordered q, k, v (q-only chunks first), so
                    # the startup critical path is just wq (2 MB) + xt
                    # (8 MB) interleaved by first need; wk/wv/wo follow.
                    _di = 0

                    def _dma(dst, src):
                        nonlocal _di
                        eng = nc.scalar if _di % 2 == 0 else nc.sync
                        _di += 1
                        eng.dma_start(dst, src)

                    def wchunk(w_sb, w_r, dc):
                        _dma(w_sb[:, dc:dc + 4, :], w_r[:, dc:dc + 4, :])

                    _dma(wq_sb[:, 0:2, :], wq_r[:, 0:2, :])
                    _dma(xts[0][:], xt_r[0])
                    _dma(wq_sb[:, 2:4, :], wq_r[:, 2:4, :])
                    _dma(xts[1][:], xt_r[1])
                    wchunk(wq_sb, wq_r, 4)
                    for do in range(2, 5):
                        _dma(xts[do][:], xt_r[do])
                    wchunk(wq_sb, wq_r, 8)
                    for do in range(5, 8):
                        _dma(xts[do][:], xt_r[do])
                    wchunk(wq_sb, wq_r, 12)
                    for do in range(8, DO):
                        _dma(xts[do][:], xt_r[do])
                    for dc in range(0, DO, 4):
                        wchunk(wk_sb, wk_r, dc)
                    for dc in range(0, DO, 4):
                        wchunk(wv_sb, wv_r, dc)
                    nc.sync.dma_start(
                        wo_sb[:], wo_d.rearrange("(cs p) d -> p cs d", p=P))

                    # 48 accumulation groups, st-major with V interleaved so
                    # the tiles phase 2 needs first (st=0 QT/KT rows + low-kb
                    # V rows) are produced first, and each xt subtile is
                    # consumed right after its DMA completes.
                    # All q groups first (chunk 0 needs only wq + xt, and
                    # 8-wide chunks consume xt tiles at ~DMA delivery pace),
                    # then k, then v; 8-wide chunks use all 8 PSUM banks.
                    groups = []
                    for st in range(NSB):
                        for ct in range(NH):
                            groups.append(("q", ct, st))
                    for st in range(NSB):
                        for ct in range(NH):
                            groups.append(("k", ct, st))
                    for sv in range(NKB):
                        groups.append(("v", sv, 0))

                    # taper the tail so the final copy burst (which gates
                    # phase 2's first PSUM-bank reuse) is short
                    chunk_sizes = [8, 8, 8, 8, 8, 4, 2, 2]
                    assert sum(chunk_sizes) == len(groups)

                    chunk_bounds = []
                    gstart = 0
                    for csz in chunk_sizes:
                        chunk_bounds.append((gstart, gstart + csz))
                        gstart += csz
                    assert gstart == len(groups)
                    for gstart, gend in chunk_bounds:
                        chunk = groups[gstart:gend]
                        psums = []
                        for kind, i0, i1 in chunk:
                            psums.append(p1ps.tile([P, SBLK], F32, tag="p1", name="p1ps"))
                        for do in range(DO):
                            for gi, (kind, i0, i1) in enumerate(chunk):
                                first = do == 0
                                last = do == DO - 1
                                if kind == "q":
                                    nc.tensor.matmul(
                                        psums[gi][:],
                                        wq_sb[:, do, ts(i0, P)],
                                        xts[do][:, ts(i1, SBLK)],
                                        start=first, stop=last)
                                elif kind == "k":
                                    nc.tensor.matmul(
                                        psums[gi][:],
                                        wk_sb[:, do, ts(i0, P)],
                                        xts[do][:, ts(i1, SBLK)],
                                        start=first, stop=last)
                                else:
                                    nc.tensor.matmul(
                                        psums[gi][:],
                                        xts[do][:, ts(i0, P)],
                                        wv_sb[:, do, :],
                                        start=first, stop=last)
                        for gi, (kind, i0, i1) in enumerate(chunk):
                            if kind == "q":
                                nc.any.tensor_copy(qt[:, i0, ts(i1, SBLK)], psums[gi][:])
                            elif kind == "k":
                                nc.any.tensor_copy(kts[i0][:, ts(i1, SBLK)], psums[gi][:])
                            else:
                                nc.any.tensor_copy(vs[i0 // 4][:, i0 % 4, :], psums[gi][:])

                # ---------------- Phase 2: causal attention ----------------
                # Software-pipelined: sc/exp for step i+1 are emitted before
                # av/rs for step i, so the PE always has an independent
                # matmul to run while ACT computes the exp it needs next.
                with tc.tile_pool(name="p2w", bufs=6) as p2w, \
                     tc.tile_pool(name="p2stat", bufs=2) as p2stat, \
                     tc.tile_pool(name="ps_s", bufs=4, space="PSUM") as ps_s, \
                     tc.tile_pool(name="ps_av", bufs=2, space="PSUM") as ps_av, \
                     tc.tile_pool(name="ps_rs", bufs=2, space="PSUM") as ps_rs:

                    steps = []
                    for qb in range(NSB):
                        nkb = 4 * (qb + 1)  # causal: only key blocks <= q blk
                        for h in range(NH):
                            for kb in range(nkb):
                                steps.append((qb, h, kb, nkb))

                    avrs = {}

                    def emit_sc(step):
                        qb, h, kb, nkb = step
                        sc = ps_s.tile([P, SBLK], F32, tag="sc")
                        probs = p2w.tile([P, SBLK], F16, tag="probs")
                        # on diagonal blocks (kb == 4*qb + a), columns
                        # q < 128a are fully masked: skip them in the scores
                        # matmul and the exp, zero those probs columns (Pool)
                        # so the full-width av/rs streams see exact zeros,
                        # and add the mask only on the 128-wide triangular
                        # block, in place on PSUM, alternating DVE/Pool
                        a = kb - 4 * qb if kb >= nkb - 4 else None
                        w0 = a * P if a else 0
                        nc.tensor.matmul(
                            sc[:, w0:],
                            kts[h][:, ts(kb, P)],
                            qt[:, h, qb * SBLK + w0:(qb + 1) * SBLK],
                            start=True, stop=True)
                        if a is not None:
                            # DVE only: GPSIMD cannot access PSUM
                            nc.vector.tensor_add(sc[:, w0:w0 + P],
                                                 sc[:, w0:w0 + P],
                                                 masks[:, a, w0:w0 + P])
                        if w0:
                            nc.gpsimd.memset(probs[:, 0:w0], 0.0)
                        nc.scalar.activation(
                            probs[:, w0:], sc[:, w0:],
                            mybir.ActivationFunctionType.Exp,
                            bias=ebias[:], scale=SCALE)
                        return probs

                    def emit_avrs(step, probs):
                        qb, h, kb, nkb = step
                        if kb == 0:
                            avt = ps_av.tile([P, SBLK], F32, tag="av", name="av")
                            rst = ps_rs.tile([P, SBLK], F32, tag="rs", name="rs")
                            avrs[(qb, h)] = (avt, rst)
                        av, rs = avrs[(qb, h)]
                        nc.tensor.matmul(
                            av[:],
                            vs[kb // 4][:, kb % 4, ts(h, P)],
                            probs[:],
                            start=(kb == 0), stop=(kb == nkb - 1))
                        nc.tensor.matmul(
                            rs[:],
                            ones[:],
                            probs[:],
                            start=(kb == 0), stop=(kb == nkb - 1))
                        if kb == nkb - 1:
                            rcp = p2stat.tile([P, SBLK], F32, tag="rcp")
                            nc.vector.reciprocal(rcp[:], rs[:])
                            nc.vector.tensor_tensor(
                                ots[qb][:, h, :], av[:], rcp[:],
                                op=mybir.AluOpType.mult)
                            del avrs[(qb, h)]

                    # depth-3 pipeline: three sc/exp stages run ahead of av/rs,
                    # covering the ~1.7us sc->mask->exp->av chain on
                    # diagonal (masked) steps
                    from collections import deque
                    pend = deque()
                    for step in steps:
                        probs = emit_sc(step)
                        pend.append((step, probs))
                        if len(pend) > 3:
                            emit_avrs(*pend.popleft())
                    while pend:
                        emit_avrs(*pend.popleft())

                # ---------------- Phase 3: output projection ----------------
                with tc.tile_pool(name="p3stage", bufs=6) as p3stage, \
                     tc.tile_pool(name="p3ps", bufs=6, space="PSUM") as p3ps:
                    # stage copies alternate DVE/ACT so copy throughput
                    # (one per ~650 ns per engine) stays ahead of the PE's
                    # 852 ns/group pace and PSUM banks recycle in time
                    for so in range(NKB):
                        for no in range(NSB):
                            po = p3ps.tile([P, SBLK], F32, tag="po")
                            for cs in range(NH):
                                nc.tensor.matmul(
                                    po[:],
                                    ots[so // 4][:, cs, ts(so % 4, P)],
                                    wo_sb[:, cs, ts(no, SBLK)],
                                    start=(cs == 0), stop=(cs == NH - 1))
                            stage = p3stage.tile([P, SBLK], F16, tag="st")
                            nc.vector.tensor_copy(stage[:], po[:])
                            deng = nc.sync if (so * NSB + no) % 2 == 0 else nc.scalar
                            deng.dma_start(
                                out_d[ts(so, P), ts(no, SBLK)], stage[:])

    nc.compile()
    return nc


def _shard_inputs(x, wq, wk, wv, wo):
    in_maps = []
    for c in range(N_CORES):
        b, g = divmod(c, NH)
        cols = slice(g * C, (g + 1) * C)
        in_maps.append({
            "xt": np.ascontiguousarray(x[b].T).astype(np.float16),
            "wq": wq[:, cols].astype(np.float16),
            "wk": wk[:, cols].astype(np.float16),
            "wv": wv[:, cols].astype(np.float16),
            "wo": np.ascontiguousarray(wo[cols, :]).astype(np.float16),
        })
    return in_maps


def kernel(x, wq, wk, wv, wo):
    from concourse.bass_utils import run_bass_kernel_spmd

    if "nc" not in _STATE:
        _STATE["nc"] = _build_kernel()
    nc = _STATE["nc"]

    in_maps = _shard_inputs(
        np.asarray(x), np.asarray(wq), np.asarray(wk),
        np.asarray(wv), np.asarray(wo))
    res = run_bass_kernel_spmd(nc, in_maps, core_ids=list(range(N_CORES)))
    out = np.zeros((B, S, D), dtype=np.float32)
    for c in range(N_CORES):
        b = c // NH
        out[b] += res.results[c]["out"].astype(np.float32)
    return out
